# revision 11
# baseline (speedup 1.0000x reference)
"""Trainium2 Bass kernel for nn_ContinuousDepthGeneModule (GNN message passing).

Strategy (8 NeuronCores, node-sharded):
  - Nodes partitioned contiguously across 8 cores (6250 real -> 6272 padded,
    49 blocks of 128).
  - Per GCN round: each core computes xw = cur @ W for its nodes (node-major,
    scaled by dinv), casts to bf16, and the [50176,128] message table is
    assembled via two AllGathers (blocks 0-23 -> tableA, 24-48 -> tableB).
    xw is produced PER SEGMENT right after that segment's gate, so the
    AllGather for the first half fires mid-round and overlaps aggregation;
    tables/bounces are double-buffered by round parity so next-round AG
    never serializes against this round's gathers.
  - Aggregation: per (segment, half) one dma_gather call pulls the per-edge
    rows; segment-sum via one-hot "S" matmuls on the TensorEngine. S is
    generated just-in-time from a tiny drel table (bf16 iota + is_equal),
    split between Vector and GpSimd engines - no S DMA from DRAM.
  - LayerNorm per node-major block: ACT computes sums/squares/apply
    (Identity with per-partition scale/bias), DVE does the tiny [128,1] math.
  - RK4 state (h0, y, cur) lives in SBUF; only acc bounces through DRAM.
  - Final global_mean_pool = per-core indicator matmul -> [G, H] partials,
    summed and normalized on host.

Self-contained: hardcodes the problem shapes; host-side prep is numpy only.
"""
import os
import sys

for _p in ("/opt/trn_rl_repo", "/root/.axon_site/_ro/trn_rl_repo"):
    if os.path.isdir(_p) and _p not in sys.path:
        sys.path.insert(0, _p)

os.environ.setdefault("MYCRO_LOCAL_CACHE", "1")

import numpy as np
import ml_dtypes
from contextlib import ExitStack

import concourse.bass as bass
import concourse.bacc as bacc
import concourse.tile as tile
from concourse import mybir
from concourse import library_config
from concourse.bass_utils import run_bass_kernel_spmd

F32 = mybir.dt.float32
BF16 = mybir.dt.bfloat16
I16 = mybir.dt.int16
AF = mybir.ActivationFunctionType
ALU = mybir.AluOpType
P = 128  # partitions


# ----------------------------------------------------------------------------
# configuration
# ----------------------------------------------------------------------------
class Cfg:
    def __init__(self, N, E, FEAT, HID, G, C, eps=1e-5, min_depth=0.1, max_depth=3.0):
        assert HID == 128, "kernel assumes hidden dim == 128"
        assert N % C == 0
        self.N, self.E, self.FEAT, self.HID, self.G, self.C = N, E, FEAT, HID, G, C
        self.eps, self.min_depth, self.max_depth = eps, min_depth, max_depth
        self.NPC = N // C                       # real nodes per core
        self.NQ = ((self.NPC + P - 1) // P) * P  # padded nodes per core
        self.B = self.NQ // P                   # dst blocks per core
        self.NTOT = self.NQ * C                 # padded total nodes
        # src-block halves (segment-aligned so the A AllGather fires mid-round)
        self.B0 = (self.B // 2 // 4) * 4        # 24 for B=49
        self.B1 = self.B - self.B0              # 25
        self.HALFA = C * P * self.B0
        self.HALFB = C * P * self.B1
        assert self.HALFA <= 32768 and self.HALFB <= 32768, "int16 gather idx limit"
        # segments of 4 dst blocks; the A half is exactly segments [0, nsegA)
        segs = []
        b = 0
        while b < self.B:
            b1 = min(b + 4, self.B)
            segs.append((b, b1))
            b = b1
        self.segments = segs
        self.nsegA = self.B0 // 4               # 6


# ----------------------------------------------------------------------------
# host-side preprocessing
# ----------------------------------------------------------------------------
def _sigmoid(x):
    return 1.0 / (1.0 + np.exp(-x))


def _pack_idx(idx):
    """int16 idx array (len % 128 == 0) -> [128, n/16] wrapped + replicated."""
    idx = np.asarray(idx, np.int16)
    n = len(idx)
    if n == 0:
        return np.zeros((128, 0), np.int16)
    arr = idx.reshape(-1, 16).T  # [16, n/16]
    return np.ascontiguousarray(np.tile(arr, (8, 1)))  # [128, n/16]


def host_prep(inputs, cfg):
    """Compute all static per-core tables + scalar constants."""
    c = cfg
    src = np.asarray(inputs["edge_index"][0], np.int64)
    dst = np.asarray(inputs["edge_index"][1], np.int64)
    batch = np.asarray(inputs["batch"], np.int64)

    # scalars
    meth_sil = float(np.mean(_sigmoid(np.asarray(inputs["methylation"], np.float64))))
    hm = _sigmoid(np.asarray(inputs["histones"], np.float64))
    act = (hm[0] + hm[2]) * 0.5
    rep = (hm[1] + hm[3]) * 0.5
    chrom = float(np.clip(act - rep + 0.5, 0.0, 1.0))
    s_scale = chrom * (1.0 - meth_sil)
    depth = float(np.clip(np.exp(float(inputs["log_depth"])), c.min_depth, c.max_depth))
    rw = float(inputs["residual_weight"])

    # gcn normalization
    deg = np.bincount(dst, minlength=c.N).astype(np.float64)
    dinv = np.where(deg > 0, 1.0 / np.sqrt(np.maximum(deg, 1.0)), 0.0).astype(np.float32)

    # edge tables
    q = dst // c.NPC
    ld = dst - q * c.NPC
    blk = ld // P
    drel = (ld % P).astype(np.float32)
    # table rows: half h rows are ordered (core, partition, block-within-half)
    qsrc = src // c.NPC
    lsrc = src % c.NPC
    psrc = lsrc % P
    bsrc = lsrc // P
    half = (bsrc >= c.B0).astype(np.int64)
    idx16 = np.where(half == 0,
                     qsrc * (P * c.B0) + psrc * c.B0 + bsrc,
                     qsrc * (P * c.B1) + psrc * c.B1 + (bsrc - c.B0))

    # group edges by (core, block, half)
    gid = ((q * c.B + blk) * 2 + half)
    order = np.argsort(gid, kind="stable")
    gid_s = gid[order]
    idx16_s = idx16[order]
    drel_s = drel[order]
    n_groups = c.C * c.B * 2
    counts = np.bincount(gid_s, minlength=n_groups).reshape(c.C, c.B, 2)
    starts = np.zeros(n_groups + 1, np.int64)
    np.cumsum(counts.reshape(-1), out=starts[1:])

    # uniform chunk counts per (block, half): max over cores
    KA = np.maximum.reduce(-(-counts[:, :, 0] // P), axis=0)  # [B]
    KB = np.maximum.reduce(-(-counts[:, :, 1] // P), axis=0)  # [B]
    offA = np.zeros(c.B + 1, np.int64)
    np.cumsum(KA, out=offA[1:])
    offB = np.zeros(c.B + 1, np.int64)
    np.cumsum(KB, out=offB[1:])
    totKA, totKB = int(offA[-1]), int(offB[-1])
    totK = totKA + totKB

    per_core = []
    for qq in range(c.C):
        iA = np.zeros(totKA * P, np.int64)
        iB = np.zeros(totKB * P, np.int64)
        dr = np.full((totK, P), -1.0, np.float32)
        # drel column layout: all A chunks (block-major) at columns offA[b]..,
        # then all B chunks at totKA + offB[b]..
        for b in range(c.B):
            for h, K_b, off, iarr, dbase in ((0, int(KA[b]), int(offA[b]), iA, int(offA[b])),
                                             (1, int(KB[b]), int(offB[b]), iB,
                                              totKA + int(offB[b]))):
                g0 = starts[(qq * c.B + b) * 2 + h]
                g1 = starts[(qq * c.B + b) * 2 + h + 1]
                cnt = g1 - g0
                iarr[off * P: off * P + cnt] = idx16_s[g0:g1]
                dcols = dr[dbase: dbase + K_b].reshape(-1)
                dcols[:cnt] = drel_s[g0:g1]
        # drel node-major for on-device one-hot generation: [128, totK] f32
        drel_nm = np.ascontiguousarray(dr.T)
        # dinv per block column
        lo, hi = qq * c.NPC, (qq + 1) * c.NPC
        dv = np.zeros(c.NQ, np.float32)
        dv[: c.NPC] = dinv[lo:hi]
        dinv_nm = np.ascontiguousarray(dv.reshape(c.B, P).T)  # [128, B]
        # pooling indicator, packed [128, B*G]
        pool = np.zeros((c.NQ, c.G), np.float32)
        bb = batch[lo:hi]
        pool[np.arange(c.NPC), bb] = 1.0
        pool_sb = np.ascontiguousarray(
            pool.reshape(c.B, P, c.G).transpose(1, 0, 2).reshape(P, c.B * c.G))
        # x feat-major + bias row
        x = np.asarray(inputs["x"], np.float32)[lo:hi]
        x_fm = np.zeros((c.FEAT + 1, c.NQ), np.float32)
        x_fm[: c.FEAT, : c.NPC] = x.T
        x_fm[c.FEAT, : c.NPC] = 1.0
        per_core.append(dict(
            x_fm=x_fm,
            idxA=_pack_idx(iA),
            idxB=_pack_idx(iB),
            drel=drel_nm,
            dinv_nm=dinv_nm,
            pool_ind=pool_sb,
        ))

    cnt_g = np.bincount(batch, minlength=c.G).astype(np.float32)

    meta = dict(
        KA=KA.astype(int).tolist(), KB=KB.astype(int).tolist(),
        offA=offA.astype(int).tolist(), offB=offB.astype(int).tolist(),
        totKA=totKA, totKB=totKB, totK=totK,
        s_scale=s_scale, dt=depth, rw=rw,
        cnt_g=cnt_g,
    )
    return per_core, meta


def _trivial(v, val):
    return bool(np.all(np.asarray(v) == val))


# ----------------------------------------------------------------------------
# kernel builder
# ----------------------------------------------------------------------------
def build_kernel(cfg, meta, wts):
    """wts: dict of weight arrays (values baked for zero/one specialization)."""
    c = cfg
    H = c.HID
    KA, KB = meta["KA"], meta["KB"]
    offA, offB = meta["offA"], meta["offB"]
    totKA, totKB, totK = meta["totKA"], meta["totKB"], meta["totK"]
    s_scale, dt, rw = meta["s_scale"], meta["dt"], meta["rw"]
    segs = c.segments
    nseg = len(segs)
    # per-segment chunk spans
    segA_n = [offA[b1] - offA[b0] for (b0, b1) in segs]
    segB_n = [offB[b1] - offB[b0] for (b0, b1) in segs]
    KAms, KBms = max(segA_n), max(segB_n)

    has_in_gb = not (_trivial(wts["in_ln_g"], 1) and _trivial(wts["in_ln_b"], 0))
    has_ln_gb = [not (_trivial(wts["ln_g"][i], 1) and _trivial(wts["ln_b"][i], 0))
                 for i in range(3)]
    has_out_gb = not (_trivial(wts["out_ln_g"], 1) and _trivial(wts["out_ln_b"], 0))
    has_gcn_b = [not _trivial(wts["gcn_b"][i], 0) for i in range(3)]
    has_out_b = not _trivial(wts["out_b"], 0)

    nc = bacc.Bacc("TRN2", target_bir_lowering=False, debug=False, num_devices=c.C,
                   num_swdge_queues=4)

    # external inputs
    x_in = nc.dram_tensor("x_fm", [c.FEAT + 1, c.NQ], F32, kind="ExternalInput")
    idxA_in = nc.dram_tensor("idxA", [128, totKA * 8], I16, kind="ExternalInput")
    idxB_in = nc.dram_tensor("idxB", [128, totKB * 8], I16, kind="ExternalInput")
    drel_in = nc.dram_tensor("drel", [128, totK], F32, kind="ExternalInput")
    dinv_in = nc.dram_tensor("dinv_nm", [128, c.B], F32, kind="ExternalInput")
    pool_in = nc.dram_tensor("pool_ind", [128, c.B * c.G], F32, kind="ExternalInput")
    inw_in = nc.dram_tensor("in_w65", [c.FEAT + 1, H], F32, kind="ExternalInput")
    w_in = [nc.dram_tensor(f"w{i}", [H, H], F32, kind="ExternalInput") for i in range(3)]
    gw1_in = nc.dram_tensor("gw1", [H, H], F32, kind="ExternalInput")
    gw2_in = nc.dram_tensor("gw2", [H, H], F32, kind="ExternalInput")
    outw_in = nc.dram_tensor("out_w", [H, H], F32, kind="ExternalInput")
    gateb_in = nc.dram_tensor("gate_b", [H, 1], F32, kind="ExternalInput")
    aux_rows = nc.dram_tensor("aux_rows", [1, 4 * H], F32, kind="ExternalInput")
    # aux_rows free-dim blocks: 0..2 gcn_b[i], 3 out_b
    lnp_in = nc.dram_tensor("ln_params", [128, 10 * H], F32, kind="ExternalInput")
    # ln_params columns: [in_g, in_b, l0_g, l0_b, l1_g, l1_b, l2_g, l2_b, out_g, out_b]
    pool_out = nc.dram_tensor("pool_out", [c.G, H], F32, kind="ExternalOutput")

    # internal DRAM (double-buffered by round parity)
    bounceA = [nc.dram_tensor(f"bounceA{p_}", [128, c.B0 * H], BF16) for p_ in range(2)]
    bounceB = [nc.dram_tensor(f"bounceB{p_}", [128, c.B1 * H], BF16) for p_ in range(2)]
    tableA = [nc.dram_tensor(f"tableA{p_}", [c.HALFA, H], BF16, addr_space="Shared")
              for p_ in range(2)]
    tableB = [nc.dram_tensor(f"tableB{p_}", [c.HALFB, H], BF16, addr_space="Shared")
              for p_ in range(2)]
    acc_dram = nc.dram_tensor("acc_dram", [128, c.NQ], F32)

    # NOTE: gpsimd cannot run tensor_scalar (InstTensorScalarPtr is in no Q7
    # library) - S one-hot generation must stay on the Vector engine.
    SPOOL_MOD = 0
    NQUE = int(os.environ.get("GNN_NQUE", "1"))

    with tile.TileContext(nc) as tc, ExitStack() as ctx:
        const = ctx.enter_context(tc.tile_pool(name="const", bufs=1))
        big = ctx.enter_context(tc.tile_pool(name="big", bufs=1))
        gp = ctx.enter_context(tc.tile_pool(name="gp", bufs=2))
        sp = ctx.enter_context(tc.tile_pool(name="sp", bufs=2))
        stb = ctx.enter_context(tc.tile_pool(name="stb", bufs=3))
        sts = ctx.enter_context(tc.tile_pool(name="sts", bufs=2))
        stc = ctx.enter_context(tc.tile_pool(name="stc", bufs=4))
        ps_agg = ctx.enter_context(tc.tile_pool(name="ps_agg", bufs=2, space="PSUM"))
        ps_tp = ctx.enter_context(tc.tile_pool(name="ps_tp", bufs=2, space="PSUM"))
        ps_seg = ctx.enter_context(tc.tile_pool(name="ps_seg", bufs=1, space="PSUM"))

        nc.gpsimd.load_library(library_config.mlp)

        # ---- constants to SBUF
        def load_const(name, src_ap, shape, dtype=F32):
            t = const.tile(shape, dtype, tag=name)
            nc.sync.dma_start(t[:], src_ap)
            return t

        iota_bf = const.tile([128, 128], BF16, tag="iota_bf")
        nc.gpsimd.iota(iota_bf[:], pattern=[[1, 128]], base=0, channel_multiplier=0,
                       allow_small_or_imprecise_dtypes=True)
        iota_f = const.tile([128, 128], F32, tag="iota_f")
        nc.gpsimd.iota(iota_f[:], pattern=[[1, 128]], base=0, channel_multiplier=0,
                       allow_small_or_imprecise_dtypes=True)
        pidx = const.tile([128, 1], F32, tag="pidx")
        nc.gpsimd.iota(pidx[:], pattern=[[0, 1]], base=0, channel_multiplier=1,
                       allow_small_or_imprecise_dtypes=True)
        ident = const.tile([128, 128], F32, tag="ident")
        nc.vector.tensor_scalar(ident[:], iota_f[:], pidx[:], None, ALU.is_equal)
        eps_t = const.tile([128, 1], F32, tag="eps")
        nc.vector.memset(eps_t[:], c.eps)
        ones_row = const.tile([1, 128], F32, tag="ones_row")
        nc.vector.memset(ones_row[:], 1.0)

        idxA = load_const("idxA", idxA_in[:, :], [128, totKA * 8], I16)
        idxB = load_const("idxB", idxB_in[:, :], [128, totKB * 8], I16)
        drel_t = load_const("drel", drel_in[:, :], [128, totK])
        dinv_nm = load_const("dinv", dinv_in[:, :], [128, c.B])
        pool_ind = load_const("pool", pool_in[:, :], [128, c.B * c.G])
        in_w65 = load_const("inw", inw_in[:, :], [c.FEAT + 1, H])
        Wt = [load_const(f"w{i}", w_in[i][:, :], [H, H]) for i in range(3)]
        gw1 = load_const("gw1", gw1_in[:, :], [H, H])
        gw2 = load_const("gw2", gw2_in[:, :], [H, H])
        out_w = load_const("outw", outw_in[:, :], [H, H])
        gate_b = load_const("gateb", gateb_in[:, :], [H, 1])
        auxr = load_const("auxr", aux_rows[:, :], [1, 4 * H])
        lnp = load_const("lnp", lnp_in[:, :], [128, 10 * H]) if (
            has_in_gb or any(has_ln_gb) or has_out_gb) else None

        # ---- persistent state (all SBUF)
        cur_fm = big.tile([128, c.NQ], F32, tag="cur")
        y_fm = big.tile([128, c.NQ], F32, tag="y")
        h0_fm = big.tile([128, c.NQ], F32, tag="h0")

        qctr = [0]   # dma_gather queue round-robin
        sctr = [0]   # S-gen engine round-robin

        # ---- helpers ------------------------------------------------------
        def ln_block(src_psum, dinv_col, gb_idx):
            """LayerNorm of one [128,128] node-major block from PSUM.

            t = src * dinv_col (per-partition, or 1.0); out = (t-mean)*rstd
            (+ g/b if gb_idx). Returns SBUF tile [128,128] f32.
            """
            t_sb = stc.tile([128, 128], F32, tag="t")
            msum = stc.tile([128, 1], F32, tag="ms")
            if dinv_col is not None:
                nc.scalar.activation(t_sb[:], src_psum, AF.Copy,
                                     scale=dinv_col, accum_out=msum[:])
            else:
                nc.scalar.activation(t_sb[:], src_psum, AF.Copy, accum_out=msum[:])
            sq = stc.tile([128, 128], F32, tag="sq")
            ssq = stc.tile([128, 1], F32, tag="ss")
            nc.scalar.activation(sq[:], t_sb[:], AF.Square, accum_out=ssq[:])
            m = stc.tile([128, 1], F32, tag="m")
            nc.vector.tensor_scalar(m[:], msum[:], 1.0 / H, None, ALU.mult)
            m2 = stc.tile([128, 1], F32, tag="m2")
            nc.vector.tensor_tensor(m2[:], m[:], m[:], ALU.mult)
            v = stc.tile([128, 1], F32, tag="v")
            nc.vector.scalar_tensor_tensor(v[:], ssq[:], 1.0 / H, m2[:],
                                           ALU.mult, ALU.subtract)
            sd = stc.tile([128, 1], F32, tag="sd")
            nc.scalar.activation(sd[:], v[:], AF.Sqrt, bias=eps_t[:])
            rstd = stc.tile([128, 1], F32, tag="rs")
            nc.vector.reciprocal(rstd[:], sd[:])
            nbias = stc.tile([128, 1], F32, tag="nb")
            nc.vector.tensor_scalar(nbias[:], m[:], -1.0, rstd[:], ALU.mult, ALU.mult)
            hnn = stb.tile([128, 128], F32, tag="hnn")
            nc.scalar.activation(hnn[:], t_sb[:], AF.Identity,
                                 scale=rstd[:], bias=nbias[:])
            if gb_idx is not None:
                g_col = lnp[:, gb_idx * 2 * H: gb_idx * 2 * H + H]
                b_col = lnp[:, gb_idx * 2 * H + H: gb_idx * 2 * H + 2 * H]
                nc.vector.tensor_tensor(hnn[:], hnn[:], g_col, ALU.mult)
                nc.vector.tensor_tensor(hnn[:], hnn[:], b_col, ALU.add)
            return hnn

        def gen_S_chunk(S_tile, local_k, gcol):
            """One-hot [128,128] for chunk at drel column gcol."""
            eng = nc.gpsimd if (SPOOL_MOD and sctr[0] % SPOOL_MOD == 0) else nc.vector
            sctr[0] += 1
            eng.tensor_scalar(S_tile[:, local_k * 128:(local_k + 1) * 128],
                              iota_bf[:], drel_t[:, gcol: gcol + 1], None,
                              ALU.is_equal)

        def seg_gather(si, par):
            """Issue gathers + S generation for segment si; returns tiles."""
            b0, b1 = segs[si]
            nA, nB = segA_n[si], segB_n[si]
            gA_t = gp.tile([128, KAms, H], BF16, tag="gA")
            gB_t = gp.tile([128, KBms, H], BF16, tag="gB")
            GW = int(os.environ.get("GNN_GW", "8"))  # chunks per gather call
            for o in range(0, nA, GW):
                w = min(GW, nA - o)
                c0 = offA[b0] + o
                nc.gpsimd.dma_gather(gA_t[:, o:o + w, :], tableA[par].ap()[:, :],
                                     idxA[:, c0 * 8: (c0 + w) * 8],
                                     w * P, w * P, H,
                                     queue_num=qctr[0] % NQUE)
                qctr[0] += 1
            for o in range(0, nB, GW):
                w = min(GW, nB - o)
                c0 = offB[b0] + o
                nc.gpsimd.dma_gather(gB_t[:, o:o + w, :], tableB[par].ap()[:, :],
                                     idxB[:, c0 * 8: (c0 + w) * 8],
                                     w * P, w * P, H,
                                     queue_num=qctr[0] % NQUE)
                qctr[0] += 1
            SA_t = sp.tile([128, KAms * 128], BF16, tag="SA")
            SB_t = sp.tile([128, KBms * 128], BF16, tag="SB")
            # generate in PE consumption order: per block, A chunks then B
            for b in range(b0, b1):
                for cc in range(KA[b]):
                    gen_S_chunk(SA_t, offA[b] - offA[b0] + cc, offA[b] + cc)
                for cc in range(KB[b]):
                    gen_S_chunk(SB_t, offB[b] - offB[b0] + cc,
                                totKA + offB[b] + cc)
            return gA_t, gB_t, SA_t, SB_t

        def agg_block(b, b0, li, tiles):
            """Accumulate chunk matmuls for dst block b -> PSUM [128,128]."""
            gA_t, gB_t, SA_t, SB_t = tiles
            tot = KA[b] + KB[b] + (1 if has_gcn_b[li] else 0)
            agg = ps_agg.tile([128, 128], F32, tag="agg")
            k = 0
            for (gbuf, S_t, base) in ((gA_t, SA_t, offA[b] - offA[b0]),
                                      (gB_t, SB_t, offB[b] - offB[b0])):
                nK = KA[b] if gbuf is gA_t else KB[b]
                for cc in range(nK):
                    sc = (base + cc) * 128
                    nc.tensor.matmul(agg[:], S_t[:, sc: sc + 128],
                                     gbuf[:, base + cc, :],
                                     start=(k == 0), stop=(k == tot - 1))
                    k += 1
            if has_gcn_b[li]:
                nc.tensor.matmul(agg[:], ones_row[:], auxr[:, li * H:(li + 1) * H],
                                 start=(k == 0), stop=True)
            return agg

        def emit_xw_seg(si, nli, par_next):
            """xw for the next round, this segment -> bounce slab DMA."""
            b0, b1 = segs[si]
            w = b1 - b0
            sxw = sts.tile([128, 4 * H], BF16, tag="sxw")
            for b in range(b0, b1):
                ps = ps_tp.tile([128, 128], F32, tag="mm", name="mm")
                nc.tensor.matmul(ps[:], cur_fm[:, b * P:(b + 1) * P], Wt[nli][:],
                                 start=True, stop=True)
                nc.scalar.activation(sxw[:, (b - b0) * H:(b - b0 + 1) * H], ps[:],
                                     AF.Copy, scale=dinv_nm[:, b: b + 1])
            if b0 < c.B0:
                nc.sync.dma_start(bounceA[par_next][:, b0 * H: b1 * H],
                                  sxw[:, : w * H])
            else:
                nc.sync.dma_start(bounceB[par_next][:, (b0 - c.B0) * H:
                                                    (b1 - c.B0) * H],
                                  sxw[:, : w * H])

        def fire_ag(si, par_next):
            if si == c.nsegA - 1:
                nc.gpsimd.collective_compute(
                    "AllGather", ALU.bypass,
                    replica_groups=[list(range(c.C))],
                    ins=[bounceA[par_next].ap().opt()],
                    outs=[tableA[par_next].ap().opt()],
                )
            elif si == nseg - 1:
                nc.gpsimd.collective_compute(
                    "AllGather", ALU.bypass,
                    replica_groups=[list(range(c.C))],
                    ins=[bounceB[par_next].ap().opt()],
                    outs=[tableB[par_next].ap().opt()],
                )

        # ---- input projection (per segment; fires round-0 AGs) -----------
        for si, (b0, b1) in enumerate(segs):
            for b in range(b0, b1):
                cols = slice(b * P, (b + 1) * P)
                xst = sts.tile([c.FEAT + 1, 128], F32, tag="xst")
                nc.sync.dma_start(xst[:], x_in[:, cols])
                ps = ps_tp.tile([128, 128], F32, tag="mm", name="mm")
                nc.tensor.matmul(ps[:], xst[:], in_w65[:], start=True, stop=True)
                hnn = ln_block(ps[:], None, 0 if has_in_gb else None)
                tp = ps_tp.tile([128, 128], F32, tag="tp")
                nc.tensor.transpose(tp[:], hnn[:], ident[:])
                # relu + epigenetic scale fused into the PSUM->SBUF copy
                nc.scalar.activation(cur_fm[:, cols], tp[:], AF.Relu,
                                     scale=float(s_scale))
            scols = slice(b0 * P, b1 * P)
            nc.vector.tensor_copy(y_fm[:, scols], cur_fm[:, scols])
            nc.vector.tensor_copy(h0_fm[:, scols], cur_fm[:, scols])
            emit_xw_seg(si, 0, 0)
            fire_ag(si, 0)

        # ---- 12 GCN rounds -------------------------------------------------
        NROUNDS = int(os.environ.get("GNN_ROUNDS", "12"))
        for r in range(NROUNDS):
            li, ki = r % 3, r // 3
            par = r % 2
            par_next = (r + 1) % 2
            nli = (r + 1) % 3
            for si, (b0, b1) in enumerate(segs):
                width = (b1 - b0) * P
                ncols = slice(b0 * P, b0 * P + width)
                tiles = seg_gather(si, par)
                if li > 0:
                    hfm_stage = sts.tile([128, 512], F32, tag="hfm", name="hfm")
                else:
                    hfm_stage = None
                for b in range(b0, b1):
                    agg = agg_block(b, b0, li, tiles)
                    hnn = ln_block(agg[:], dinv_nm[:, b: b + 1],
                                   (1 + li) if has_ln_gb[li] else None)
                    tp = ps_tp.tile([128, 128], F32, tag="tp")
                    nc.tensor.transpose(tp[:], hnn[:], ident[:])
                    if li == 0:
                        nc.scalar.activation(cur_fm[:, b * P:(b + 1) * P], tp[:],
                                             AF.Copy)
                    else:
                        nc.scalar.activation(
                            hfm_stage[:, (b - b0) * P:(b - b0 + 1) * P], tp[:],
                            AF.Copy)
                if li > 0:
                    gps = ps_seg.tile([128, 512], F32, tag="g5")
                    nc.tensor.matmul(gps[:, :width], gw1[:], cur_fm[:, ncols],
                                     start=True, stop=False)
                    nc.tensor.matmul(gps[:, :width], gw2[:], hfm_stage[:, :width],
                                     start=False, stop=True)
                    g_sb = sts.tile([128, 512], F32, tag="sw")
                    nc.scalar.activation(g_sb[:, :width], gps[:, :width],
                                         AF.Sigmoid, bias=gate_b[:])
                    d_sb = sts.tile([128, 512], F32, tag="sw")
                    nc.vector.tensor_tensor(d_sb[:, :width], hfm_stage[:, :width],
                                            cur_fm[:, ncols], ALU.subtract)
                    nc.vector.tensor_tensor(d_sb[:, :width], g_sb[:, :width],
                                            d_sb[:, :width], ALU.mult)
                    nc.vector.tensor_tensor(cur_fm[:, ncols], cur_fm[:, ncols],
                                            d_sb[:, :width], ALU.add)
                if li == 2:
                    # RK4 stage boundary
                    wk = [1.0, 2.0, 2.0, 1.0][ki]
                    cy = [dt / 2, dt / 2, dt, 0.0][ki]
                    tnh = sts.tile([128, 512], F32, tag="sw")
                    nc.scalar.activation(tnh[:, :width], cur_fm[:, ncols], AF.Tanh)
                    kst = sts.tile([128, 512], F32, tag="sw")
                    nc.vector.scalar_tensor_tensor(kst[:, :width], y_fm[:, ncols],
                                                   rw, tnh[:, :width],
                                                   ALU.mult, ALU.add)
                    if ki == 0:
                        nc.sync.dma_start(acc_dram[:, ncols], kst[:, :width])
                    else:
                        ast = sts.tile([128, 512], F32, tag="sw")
                        nc.sync.dma_start(ast[:, :width], acc_dram[:, ncols])
                        nc.vector.scalar_tensor_tensor(ast[:, :width],
                                                       kst[:, :width], wk,
                                                       ast[:, :width],
                                                       ALU.mult, ALU.add)
                        if ki < 3:
                            nc.sync.dma_start(acc_dram[:, ncols], ast[:, :width])
                    if ki < 3:
                        nc.vector.scalar_tensor_tensor(cur_fm[:, ncols],
                                                       kst[:, :width], cy,
                                                       h0_fm[:, ncols],
                                                       ALU.mult, ALU.add)
                        nc.vector.tensor_copy(y_fm[:, ncols], cur_fm[:, ncols])
                    else:
                        nc.vector.scalar_tensor_tensor(cur_fm[:, ncols],
                                                       ast[:, :width], dt / 6.0,
                                                       h0_fm[:, ncols],
                                                       ALU.mult, ALU.add)
                if r < 11:
                    emit_xw_seg(si, nli, par_next)
                    fire_ag(si, par_next)

        # ---- output projection + pooling ----------------------------------
        pool_ps = ps_seg.tile([c.G, H], F32, tag="pool", bufs=1)
        for b in range(c.B):
            cols = slice(b * P, (b + 1) * P)
            ps = ps_tp.tile([128, 128], F32, tag="mm", name="mm")
            nc.tensor.matmul(ps[:], cur_fm[:, cols], out_w[:], start=True,
                             stop=not has_out_b)
            if has_out_b:
                nc.tensor.matmul(ps[:], ones_row[:], auxr[:, 3 * H: 4 * H],
                                 start=False, stop=True)
            hnn = ln_block(ps[:], None, 4 if has_out_gb else None)
            nc.tensor.matmul(pool_ps[:], pool_ind[:, b * c.G:(b + 1) * c.G], hnn[:],
                             start=(b == 0), stop=(b == c.B - 1))
        pool_sb = stb.tile([c.G, H], F32, tag="po")
        nc.vector.tensor_copy(pool_sb[:], pool_ps[:])
        nc.sync.dma_start(pool_out[:, :], pool_sb[:])

    return nc


# ----------------------------------------------------------------------------
# entry point
# ----------------------------------------------------------------------------
_CACHE = {}
LAST_EXEC_NS = None
LAST_RESULTS = None


def _weights_pack(inputs, cfg):
    c = cfg
    in_w = np.asarray(inputs["in_w"], np.float32)
    in_b = np.asarray(inputs["in_b"], np.float32)
    in_w65 = np.concatenate([in_w, in_b[None, :]], axis=0)
    gate_w = np.asarray(inputs["gate_w"], np.float32)
    aux = np.zeros((1, 4 * c.HID), np.float32)
    aux[0, : 3 * c.HID] = np.asarray(inputs["gcn_b"], np.float32).reshape(-1)
    aux[0, 3 * c.HID:] = np.asarray(inputs["out_b"], np.float32)
    lnp = np.zeros((128, 10 * c.HID), np.float32)
    seq = [inputs["in_ln_g"], inputs["in_ln_b"],
           inputs["ln_g"][0], inputs["ln_b"][0],
           inputs["ln_g"][1], inputs["ln_b"][1],
           inputs["ln_g"][2], inputs["ln_b"][2],
           inputs["out_ln_g"], inputs["out_ln_b"]]
    for i, v in enumerate(seq):
        lnp[:, i * c.HID:(i + 1) * c.HID] = np.asarray(v, np.float32)[None, :]
    return dict(
        in_w65=in_w65,
        w=[np.ascontiguousarray(np.asarray(inputs["gcn_w"], np.float32)[i])
           for i in range(3)],
        gw1=np.ascontiguousarray(gate_w[: c.HID]),
        gw2=np.ascontiguousarray(gate_w[c.HID:]),
        out_w=np.asarray(inputs["out_w"], np.float32),
        gate_b=np.asarray(inputs["gate_b"], np.float32).reshape(c.HID, 1),
        aux_rows=aux,
        ln_params=lnp,
        # raw (for specialization flags)
        in_ln_g=inputs["in_ln_g"], in_ln_b=inputs["in_ln_b"],
        ln_g=np.asarray(inputs["ln_g"]), ln_b=np.asarray(inputs["ln_b"]),
        out_ln_g=inputs["out_ln_g"], out_ln_b=inputs["out_ln_b"],
        gcn_b=np.asarray(inputs["gcn_b"]), out_b=inputs["out_b"],
    )


def kernel_impl(inputs, cfg, profile=False):
    global LAST_EXEC_NS, LAST_RESULTS
    inputs = {k: np.asarray(v) for k, v in inputs.items()}
    per_core, meta = host_prep(inputs, cfg)
    wts = _weights_pack(inputs, cfg)

    key = (cfg.N, cfg.E, cfg.C,
           hash(inputs["edge_index"].tobytes()),
           hash(inputs["batch"].tobytes()),
           meta["s_scale"], meta["dt"], meta["rw"])
    if key not in _CACHE:
        nc = build_kernel(cfg, meta, wts)
        if not nc.is_finalized():
            nc.finalize()
        _CACHE.clear()
        _CACHE[key] = nc
    nc = _CACHE[key]

    in_maps = []
    for q in range(cfg.C):
        m = dict(per_core[q])
        m["in_w65"] = wts["in_w65"]
        for i in range(3):
            m[f"w{i}"] = wts["w"][i]
        m["gw1"] = wts["gw1"]
        m["gw2"] = wts["gw2"]
        m["out_w"] = wts["out_w"]
        m["gate_b"] = wts["gate_b"]
        m["aux_rows"] = wts["aux_rows"]
        m["ln_params"] = wts["ln_params"]
        in_maps.append(m)

    res = run_bass_kernel_spmd(nc, in_maps, core_ids=list(range(cfg.C)),
                               trace=profile)
    LAST_RESULTS = res
    LAST_EXEC_NS = res.exec_time_ns

    pooled = np.zeros((cfg.G, cfg.HID), np.float64)
    for q in range(cfg.C):
        pooled += np.asarray(res.results[q]["pool_out"], np.float64)
    cnt = np.maximum(meta["cnt_g"], 1.0)
    out = (pooled / cnt[:, None]).astype(np.float32)
    return out


def kernel(**inputs):
    cfg = Cfg(N=50000, E=800000, FEAT=64, HID=128, G=8, C=8)
    profile = bool(int(os.environ.get("GNN_PROFILE", "0")))
    return kernel_impl(inputs, cfg, profile=profile)
